# revision 2
# baseline (speedup 1.0000x reference)
"""AutoregressiveRAM kernel for trn2 — v3.

Changes vs v2:
- pos-bit contribution folded into PADD[t mod 16] (precomputed f32 table):
  no pos windows, no POSW, scalar engine out of the per-step loop.
- G1 AND+reduce runs on u32 pairs (DVE is 1 elem/cyc regardless of width,
  so pairing halves the dominant stage). W kept even for pairing.
- G2 one-hots in [s, w] layout (inner-dim stride-0 broadcast) so all
  reduces are contiguous (transposed reduces measured ~3x slower).
- DVE casts psum->swords and addr->addrI itself (engine chain DVE<->PE only).
"""
import sys
sys.path.insert(0, '/opt/trn_rl_repo')
import numpy as np

BITS = 8192
NB_T = 10
POS = 4
P = 128
S = 64
HI_W = 64
NSTEP = 4096
KOUT = NSTEP // 16

_CACHE = {}


def _color_and_place(conn, J, W, seed=0):
    """Pure-state greedy rainbow coloring + slot placement (pos taps excluded).

    Returns dict with color[8192], slot_k, slot_n, wordcol[8192], bitpos[8192].
    Window j = swords cols [j*W, (j+1)*W); pack column c maps to word
    (g=(c//64)%8, n=c%64): bits at partitions 16g..16g+15, column n.
    """
    rng = np.random.default_rng(seed)
    nbrs = [[] for _ in range(BITS)]
    for n in range(BITS):
        row = [b for b in conn[n] if b < BITS]
        for a in row:
            for b in row:
                if a != b:
                    nbrs[a].append(b)
    nbrs = [np.unique(np.array(x, dtype=np.int32)) if x else np.zeros(0, np.int32)
            for x in nbrs]

    C_s = J * W
    cover = [[] for _ in range(8 * 64)]
    col_of = {}
    for c in range(C_s):
        g, n = (c // 64) % 8, c % 64
        j = c // W
        cover[g * 64 + n].append(j)
        col_of[(g, n, j)] = c
    free = [[] for _ in range(J)]
    for g in range(8):
        for n in range(64):
            for j in cover[g * 64 + n]:
                for r in range(16):
                    free[j].append((16 * g + r, n))
    taken = np.zeros((P, 64), bool)
    free_ptr = [0] * J

    deg = np.array([len(x) for x in nbrs])
    order = np.argsort(-deg, kind='stable')
    color = np.full(BITS, -1, np.int32)
    slot_k = np.full(BITS, -1, np.int32)
    slot_n = np.full(BITS, -1, np.int32)
    wordcol = np.full(BITS, -1, np.int32)
    bitpos = np.full(BITS, -1, np.int32)
    cnt = np.zeros(J, np.int64)
    for q in order:
        forb = set(color[nbrs[q]].tolist()) if len(nbrs[q]) else set()
        forb.discard(-1)
        best, best_load = -1, None
        for j in rng.permutation(J):
            if j in forb:
                continue
            fl = free[j]
            while free_ptr[j] < len(fl) and taken[fl[free_ptr[j]][0], fl[free_ptr[j]][1]]:
                free_ptr[j] += 1
            if free_ptr[j] >= len(fl):
                continue
            if best == -1 or cnt[j] < best_load:
                best, best_load = j, cnt[j]
        if best == -1:
            return None
        j = best
        k, n = free[j][free_ptr[j]]
        free_ptr[j] += 1
        taken[k, n] = True
        color[q] = j
        slot_k[q] = k
        slot_n[q] = n
        wordcol[q] = col_of[(k // 16, n, j)]
        bitpos[q] = k % 16
        cnt[j] += 1

    if (color == -1).any():
        return None
    return dict(color=color, slot_k=slot_k, slot_n=slot_n,
                wordcol=wordcol, bitpos=bitpos)


def _build_constants(transition_memory, transition_connections, J, W, place):
    conn = transition_connections
    color = place['color']; slot_k = place['slot_k']; slot_n = place['slot_n']
    wordcol = place['wordcol']; bitpos = place['bitpos']
    C_s = -(-(J * W) // 64) * 64
    if C_s <= 512:
        C_s = 576

    neuron_at = np.full((P, S), -1, np.int64)
    neuron_at[slot_k, slot_n] = np.arange(BITS)
    assert (neuron_at >= 0).all()

    # tap descriptors for on-device expansion of ANDMASK / POWC
    TAPIDX = np.full((P, NB_T, S), J * W, np.uint16)   # jw index, OOR -> no-op
    TAPVAL = np.zeros((P, NB_T, S), np.uint16)         # 1 << bitpos
    TAPJ = np.full((P, NB_T, S), J, np.uint16)         # color, OOR -> no-op
    TAPW = np.zeros((P, NB_T, S), np.float64)          # 2^(9-jt) / 2^bitpos
    PADD = np.zeros((P, 16, S), np.float64)
    sk = slot_k; sn = slot_n
    for jt in range(NB_T):
        b = conn[:, jt]                                # [BITS] tap of each neuron
        wt = float(2 ** (NB_T - 1 - jt))
        is_pos = b >= BITS
        # pos taps -> PADD
        qpos = np.nonzero(is_pos)[0]
        for q in qpos:
            i = int(b[q]) - BITS
            for m in range(16):
                PADD[sk[q], m, sn[q]] += wt * ((m >> (3 - i)) & 1)
        qst = np.nonzero(~is_pos)[0]
        bb = b[qst]
        TAPIDX[sk[qst], jt, sn[qst]] = wordcol[bb]
        TAPVAL[sk[qst], jt, sn[qst]] = (1 << bitpos[bb]).astype(np.uint16)
        TAPJ[sk[qst], jt, sn[qst]] = color[bb]
        TAPW[sk[qst], jt, sn[qst]] = wt / (2.0 ** bitpos[bb])
    assert np.array_equal(TAPW.astype(np.float32).astype(np.float64), TAPW)

    cidx = np.arange(C_s)
    gofc = (cidx // 64) % 8
    MASKG = (np.arange(P)[:, None] // 16 == gofc[None, :]).astype(np.float32)
    PACKW = np.tile((2.0 ** (np.arange(P) % 16))[:, None], (1, P)).astype(np.float32)

    # TBLS u16 [P, S, HI_W]: packed table words in [s, w] layout
    T = transition_memory
    Tb = (T > 0.5).astype(np.uint16).reshape(BITS, HI_W, 16)
    U16 = (Tb << np.arange(16, dtype=np.uint16)[None, None, :]).sum(-1).astype(np.uint16)
    TBLS = U16[neuron_at, :]                    # [P, S, 64]

    return dict(TAPIDX=TAPIDX, TAPVAL=TAPVAL, TAPJ=TAPJ,
                TAPW=TAPW.astype(np.float32),
                PADD=PADD.astype(np.float32), MASKG=MASKG, PACKW=PACKW,
                TBLS=TBLS, C_s=C_s, neuron_at=neuron_at)


def _f32c(u16arr):
    a = np.ascontiguousarray(u16arr.reshape(P, -1))
    assert a.shape[1] % 2 == 0
    return a.view(np.float32)


def _bf16c(f32arr):
    import ml_dtypes
    a = np.ascontiguousarray(f32arr.reshape(P, -1)).astype(ml_dtypes.bfloat16)
    assert a.shape[1] % 2 == 0
    return a.view(np.uint16).view(np.float32)


def _build_program(J, W, C_s, n_steps=NSTEP):
    from concourse import bacc, mybir
    kout = n_steps // 16
    SJ = S * J
    JW = J * W
    assert W % 2 == 0
    nc = bacc.Bacc('TRN2', target_bir_lowering=False, debug=False)
    dt = mybir.dt

    def param(name, cols, dtype=dt.float32):
        return nc.declare_dram_parameter(name, [P, cols], dtype, isOutput=False)

    x_tapidx = param('x_tapidx', (NB_T * S) // 2)
    x_tapval = param('x_tapval', (NB_T * S) // 2)
    x_tapj = param('x_tapj', (NB_T * S) // 2)
    x_tapw = param('x_tapw', (NB_T * S) // 2)   # bf16 carrier
    x_padd = param('x_padd', 16 * S)
    x_packw = param('x_packw', P // 2)
    x_tbls = param('x_tbls', (S * HI_W) // 2)
    x_state0 = param('x_state0', S // 2)
    y = nc.declare_dram_parameter('y', [kout, BITS], dt.uint16, isOutput=True)

    from contextlib import ExitStack
    es = ExitStack()
    block = es.enter_context(nc.Block())
    sb = lambda name, cols, dtype: es.enter_context(nc.sbuf_tensor(name, [P, cols], dtype))
    ANDMASK = sb('ANDMASK', S * JW, dt.uint16)
    POWC = sb('POWC', SJ, dt.float32)
    PADD = sb('PADD', 16 * S, dt.float32)
    MASKG = sb('MASKG', C_s, dt.bfloat16)
    PACKW = sb('PACKW', P, dt.bfloat16)
    TAPIDX = sb('TAPIDX', NB_T * S, dt.uint16)
    TAPVAL = sb('TAPVAL', NB_T * S, dt.uint16)
    TAPJ = sb('TAPJ', NB_T * S, dt.uint16)
    TAPW = sb('TAPW', NB_T * S, dt.bfloat16)
    IOTAJW = sb('IOTAJW', JW, dt.uint16)
    IOTAJ = sb('IOTAJ', J, dt.uint16)
    GIOTA = sb('GIOTA', C_s, dt.uint16)
    PIDX = sb('PIDX', 2, dt.uint16)
    PIDXF = sb('PIDXF', 2, dt.float32)
    TBLS = sb('TBLS', S * HI_W, dt.uint16)
    IOTA_WS = sb('IOTA_WS', S * HI_W, dt.uint16)
    IOTA16S = sb('IOTA16S', S * 16, dt.uint16)
    POW2S = sb('POW2S', S * 16, dt.uint16)
    swords = sb('swords', JW, dt.uint16)
    bitsrep = sb('bitsrep', C_s, dt.bfloat16)
    tmp1 = sb('tmp1', S * JW, dt.uint16)
    sel32 = sb('sel32', 2 * SJ, dt.uint16)
    addr = sb('addr', S, dt.float32)
    addrI = sb('addrI', S, dt.uint16)
    hi6 = sb('hi6', S, dt.uint16)
    lo4 = sb('lo4', S, dt.uint16)
    word2 = sb('word2', S, dt.uint16)
    pw = sb('pw', S, dt.uint16)
    tb = sb('tb', S, dt.uint16)
    bitu = sb('bitu', S, dt.uint16)
    acc = sb('acc', S, dt.uint16)
    acc2 = sb('acc2', S, dt.uint16)
    stage = sb('stage', S, dt.uint16)
    psum = es.enter_context(nc.psum_tensor('psum', [P, C_s], dt.float32))

    s_in = es.enter_context(nc.semaphore('s_in'))
    s_cg = es.enter_context(nc.semaphore('s_cg'))
    s_rhs = es.enter_context(nc.semaphore('s_rhs'))
    s_pe = es.enter_context(nc.semaphore('s_pe'))
    s_bf = es.enter_context(nc.semaphore('s_bf'))
    s_dma = es.enter_context(nc.semaphore('s_dma'))

    N_IN_DMAS = 8

    AND = mybir.AluOpType.bitwise_and
    OR = mybir.AluOpType.bitwise_or
    MULT = mybir.AluOpType.mult
    ADD = mybir.AluOpType.add
    EQ = mybir.AluOpType.is_equal
    SHR = mybir.AluOpType.logical_shift_right
    SHL = mybir.AluOpType.logical_shift_left
    X = mybir.AxisListType.X

    import concourse.bass as bass_mod
    ds = bass_mod.ds

    @block.sync
    def _(sync):
        for t_, src in [(TAPIDX, x_tapidx), (TAPVAL, x_tapval), (TAPJ, x_tapj),
                        (TAPW, x_tapw), (PADD, x_padd), (PACKW, x_packw),
                        (TBLS, x_tbls), (bitu, x_state0)]:
            sync.dma_start(out=t_[:], in_=src[:].bitcast(t_.dtype)).then_inc(s_in, 16)
        with sync.Fori(0, kout) as k:
            sync.wait_ge(s_bf, k + 1)
            sync.dma_start(
                out=y[ds(k, 1), :],
                in_=stage[:].unsqueeze(1),
            ).then_inc(s_dma, 16)

    @block.gpsimd
    def _(gpsimd):
        gpsimd.iota(IOTA_WS[:].rearrange('p (s w) -> p s w', s=S),
                    pattern=[[0, S], [1, HI_W]], base=0, channel_multiplier=0)
        gpsimd.iota(IOTA16S[:].rearrange('p (s b) -> p s b', s=S),
                    pattern=[[0, S], [1, 16]], base=0, channel_multiplier=0)
        gpsimd.iota(IOTAJW[:], pattern=[[1, JW]], base=0, channel_multiplier=0)
        gpsimd.iota(IOTAJ[:], pattern=[[1, J]], base=0, channel_multiplier=0)
        gpsimd.iota(GIOTA[:].rearrange('p (a b) -> p a b', b=64),
                    pattern=[[1, C_s // 64], [0, 64]], base=0, channel_multiplier=0)
        gpsimd.iota(PIDX[:], pattern=[[0, 2]], base=0, channel_multiplier=1)
        gpsimd.memset(POW2S[:], 1)
        gpsimd.memset(acc[:], 0)
        gpsimd.memset(ANDMASK[:], 0)
        gpsimd.memset(POWC[:], 0)
        gpsimd.drain().then_inc(s_cg, 1)

    @block.tensor
    def _(tensor):
        tensor.wait_ge(s_in, 16 * N_IN_DMAS)
        with tensor.Fori(0, n_steps, 16) as base:
            for u in range(16):
                tensor.wait_ge(s_rhs, base + (u + 1))
                tensor.matmul(psum[:, 0:512], PACKW[:], bitsrep[:, 0:512],
                              start=True, stop=True)
                tensor.matmul(psum[:, 512:C_s], PACKW[:], bitsrep[:, 512:C_s],
                              start=True, stop=True).then_inc(s_pe, 1)

    @block.vector
    def _(vector):
        _lp = nc.allow_low_precision(reason='u16/u32 one-hot reductions, exact by construction')
        _lp.__enter__()
        vector.wait_ge(s_in, 16 * N_IN_DMAS)
        vector.wait_ge(s_cg, 1)
        vector.tensor_tensor(POW2S[:], POW2S[:], IOTA16S[:], SHL)
        vector.drain()
        # MASKG = (p//16 == (c//64)%8) in bf16
        vector.tensor_scalar(GIOTA[:], GIOTA[:], 7, None, AND)
        vector.tensor_scalar(PIDX[:], PIDX[:], 4, None, SHR)
        vector.drain()
        vector.tensor_scalar(PIDXF[:], PIDX[:], 0, None, ADD)
        vector.drain()
        vector.tensor_scalar(MASKG[:], GIOTA[:], PIDXF[:, 0:1], None, EQ)
        vector.drain()
        # expand ANDMASK from tap descriptors
        tmp1v = tmp1[:].rearrange('p (s c) -> p s c', s=S)
        for t in range(NB_T):
            vector.tensor_tensor(
                tmp1v,
                TAPIDX[:].rearrange('p (t s) -> p t s', t=NB_T)[:, t, :].unsqueeze(2).broadcast_to([P, S, JW]),
                IOTAJW[:].unsqueeze(1).broadcast_to([P, S, JW]),
                EQ)
            vector.drain()
            vector.tensor_tensor(
                tmp1v, tmp1v,
                TAPVAL[:].rearrange('p (t s) -> p t s', t=NB_T)[:, t, :].unsqueeze(2).broadcast_to([P, S, JW]),
                MULT)
            vector.drain()
            vector.tensor_tensor(ANDMASK[:].rearrange('p (s c) -> p s c', s=S),
                                 ANDMASK[:].rearrange('p (s c) -> p s c', s=S),
                                 tmp1v, OR)
            vector.drain()
        # expand POWC from tap descriptors (reuses tmp1 scratch; one-hot in f32
        # so the TAPW multiply is a f32 x bf16 op)
        ohjf = tmp1[:, 0:2 * SJ].bitcast(dt.float32).rearrange('p (s j) -> p s j', s=S)
        tmpf = tmp1[:, 2 * SJ:4 * SJ].bitcast(dt.float32).rearrange('p (s j) -> p s j', s=S)
        for t in range(NB_T):
            vector.tensor_tensor(
                ohjf,
                TAPJ[:].rearrange('p (t s) -> p t s', t=NB_T)[:, t, :].unsqueeze(2).broadcast_to([P, S, J]),
                IOTAJ[:].unsqueeze(1).broadcast_to([P, S, J]),
                EQ)
            vector.drain()
            vector.tensor_tensor(
                tmpf, ohjf,
                TAPW[:].rearrange('p (t s) -> p t s', t=NB_T)[:, t, :].unsqueeze(2).broadcast_to([P, S, J]),
                MULT)
            vector.drain()
            vector.tensor_tensor(POWC[:].rearrange('p (s j) -> p s j', s=S),
                                 POWC[:].rearrange('p (s j) -> p s j', s=S),
                                 tmpf, ADD)
            vector.drain()
        # prologue: bitsrep for it=1 from state0 (in bitu)
        vector.tensor_tensor(bitsrep[:].rearrange('p (r n) -> p r n', n=S),
                             bitu[:].unsqueeze(1).broadcast_to([P, C_s // S, S]),
                             MASKG[:].rearrange('p (r n) -> p r n', n=S),
                             MULT).then_inc(s_rhs, 1)
        vector.drain()

        W2 = W // 2
        tmp32 = tmp1[:].bitcast(dt.uint32)       # [P, S*JW/2] u32
        sel32v = sel32[:].bitcast(dt.uint32)     # [P, SJ] u32
        sw32 = swords[:].bitcast(dt.uint32)      # [P, JW/2]
        am32 = ANDMASK[:].bitcast(dt.uint32)
        # scratch carved from tmp1 (free once the G1 reduce lands in sel32)
        o = 0
        selh = tmp1[:, o:o + 2 * SJ]; o += 2 * SJ
        selhv = selh.bitcast(dt.uint32)
        sel2 = tmp1[:, o:o + SJ]; o += SJ
        if o % 2:
            o += 1
        addrt = tmp1[:, o:o + 2 * SJ].bitcast(dt.float32); o += 2 * SJ
        onehot = tmp1[:, o:o + S * HI_W]; o += S * HI_W
        wtmp = tmp1[:, o:o + S * HI_W]; o += S * HI_W
        oh16 = tmp1[:, o:o + S * 16]; o += S * 16
        ptmp = tmp1[:, o:o + S * 16]; o += S * 16
        assert o <= S * JW, (o, S * JW)

        with vector.Fori(0, n_steps, 16) as base:
            for u in range(16):
                m = (u + 1) & 15
                vector.wait_ge(s_pe, base + (u + 1))
                # cast psum words -> u16 swords
                vector.tensor_scalar(swords[:], psum[:, 0:JW], 0, None, ADD)
                vector.drain()
                # G1 on u32 pairs
                vector.tensor_tensor(
                    tmp32.rearrange('p (s j c) -> p s j c', s=S, j=J),
                    sw32.rearrange('p (j c) -> p j c', j=J).unsqueeze(1).broadcast_to([P, S, J, W2]),
                    am32.rearrange('p (s j c) -> p s j c', s=S, j=J),
                    AND)
                vector.drain()
                vector.tensor_reduce(sel32v.rearrange('p (s j) -> p s j', s=S),
                                     tmp32.rearrange('p (s j c) -> p s j c', s=S, j=J),
                                     axis=X, op=ADD)
                vector.drain()
                # merge u16 lanes: sel2 = lo + hi (exactly one is nonzero)
                vector.tensor_scalar(selhv, sel32v, 16, None, SHR)
                vector.drain()
                vector.tensor_tensor(selhv, sel32v, selhv, ADD)
                vector.drain()
                vector.tensor_scalar(selhv, selhv, 0xFFFF, None, AND)
                vector.drain()
                vector.tensor_scalar(sel2, selhv, 0, None, ADD)  # u32 -> u16
                vector.drain()
                # addr = sum_j sel2 * POWC + PADD[m]
                vector.tensor_tensor(addrt.rearrange('p (s j) -> p s j', s=S),
                                     sel2.rearrange('p (s j) -> p s j', s=S),
                                     POWC[:].rearrange('p (s j) -> p s j', s=S),
                                     MULT)
                vector.drain()
                vector.tensor_reduce(addr[:].rearrange('p s -> p s'),
                                     addrt.rearrange('p (s j) -> p s j', s=S),
                                     axis=X, op=ADD)
                vector.drain()
                vector.tensor_tensor(addr[:], addr[:],
                                     PADD[:].rearrange('p (m s) -> p m s', m=16)[:, m, :],
                                     ADD)
                vector.drain()
                vector.tensor_scalar(addrI[:], addr[:], 0, None, ADD)  # f32 -> u16
                vector.drain()
                vector.tensor_scalar(hi6[:], addrI[:], 4, None, SHR)
                vector.tensor_scalar(lo4[:], addrI[:], 15, None, AND)
                vector.drain()
                # G2 in [s, w] layout, contiguous reduces
                vector.tensor_tensor(onehot.rearrange('p (s w) -> p s w', s=S),
                                     hi6[:].unsqueeze(2).broadcast_to([P, S, HI_W]),
                                     IOTA_WS[:].rearrange('p (s w) -> p s w', s=S),
                                     EQ)
                vector.tensor_tensor(oh16.rearrange('p (s b) -> p s b', s=S),
                                     lo4[:].unsqueeze(2).broadcast_to([P, S, 16]),
                                     IOTA16S[:].rearrange('p (s b) -> p s b', s=S),
                                     EQ)
                vector.drain()
                vector.tensor_tensor(wtmp.rearrange('p (s w) -> p s w', s=S),
                                     onehot.rearrange('p (s w) -> p s w', s=S),
                                     TBLS[:].rearrange('p (s w) -> p s w', s=S),
                                     MULT)
                vector.tensor_tensor(ptmp.rearrange('p (s b) -> p s b', s=S),
                                     oh16.rearrange('p (s b) -> p s b', s=S),
                                     POW2S[:].rearrange('p (s b) -> p s b', s=S),
                                     MULT)
                vector.drain()
                vector.tensor_reduce(word2[:].rearrange('p s -> p s'),
                                     wtmp.rearrange('p (s w) -> p s w', s=S),
                                     axis=X, op=ADD)
                vector.tensor_reduce(pw[:].rearrange('p s -> p s'),
                                     ptmp.rearrange('p (s b) -> p s b', s=S),
                                     axis=X, op=ADD)
                vector.drain()
                vector.tensor_tensor(tb[:], word2[:], pw[:], AND)
                vector.drain()
                vector.tensor_scalar(bitu[:], tb[:], 0, None, mybir.AluOpType.is_gt)
                vector.drain()
                # next-iteration rhs (unblocks PE) + acc update
                vector.tensor_tensor(bitsrep[:].rearrange('p (r n) -> p r n', n=S),
                                     bitu[:].unsqueeze(1).broadcast_to([P, C_s // S, S]),
                                     MASKG[:].rearrange('p (r n) -> p r n', n=S),
                                     MULT).then_inc(s_rhs, 1)
                vector.tensor_scalar(acc2[:], acc[:], 0x7fff, 1, AND, SHL)
                vector.drain()
                vector.tensor_tensor(acc[:], acc2[:], bitu[:], OR)
                vector.drain()
            vector.wait_ge(s_dma, base)
            vector.tensor_scalar(stage[:], acc[:], 0, None, ADD).then_inc(s_bf, 1)
            vector.drain()
        _lp.__exit__(None, None, None)

    es.close()
    nc.finalize()
    return nc


def kernel(transition_memory, initial_memory, transition_connections,
           initial_connections, length):
    from concourse.bass_utils import run_bass_kernel_spmd
    import time as _time
    import os as _os
    _dbg = _os.environ.get('KERNEL_TIMERS')
    length = int(length)
    conn = np.asarray(transition_connections)
    if 'prog' not in _CACHE:
        place = None
        for (J, W, seed) in [(38, 14, 0), (39, 14, 1), (40, 14, 0), (42, 14, 0)]:
            place = _color_and_place(conn, J, W, seed=seed)
            if place is not None:
                break
        assert place is not None, 'coloring failed'
        consts = _build_constants(np.asarray(transition_memory), conn, J, W, place)
        prog = _build_program(J, W, consts['C_s'], NSTEP)
        _CACHE['prog'] = (J, W, place, consts, prog)
    J, W, place, consts, prog = _CACHE['prog']

    # step 0 host-side
    ic = np.asarray(initial_connections)
    im = np.asarray(initial_memory)
    addr0 = (np.zeros(POS, np.int64)[ic] * (2 ** np.arange(3, -1, -1))).sum(1)
    out0 = im[np.arange(BITS), addr0].astype(np.float32)
    st0 = (out0 > 0.5).astype(np.uint16)[consts['neuron_at']]

    _tp = _time.perf_counter()
    if 'ins' not in _CACHE:
        _CACHE['ins'] = {
            'x_tapidx': _f32c(consts['TAPIDX']),
            'x_tapval': _f32c(consts['TAPVAL']),
            'x_tapj': _f32c(consts['TAPJ']),
            'x_tapw': _bf16c(consts['TAPW']),
            'x_padd': np.ascontiguousarray(consts['PADD'].reshape(P, -1)),
            'x_packw': _bf16c(consts['PACKW']),
            'x_tbls': _f32c(consts['TBLS']),
            'x_state0': _f32c(st0),
        }
    ins = _CACHE['ins']
    if _dbg:
        print(f"[kt] ins prep {_time.perf_counter()-_tp:.3f}s", flush=True)
    _t0 = _time.perf_counter()
    res = run_bass_kernel_spmd(prog, [ins], core_ids=[0])
    global LAST_EXEC_NS
    LAST_EXEC_NS = res.exec_time_ns
    if LAST_EXEC_NS is None:
        LAST_EXEC_NS = int((_time.perf_counter() - _t0) * 1e9)
    if _dbg:
        print(f"[kt] run_bass {_time.perf_counter()-_t0:.3f}s", flush=True)
    _tp = _time.perf_counter()
    dev = np.array(res.results[0]['y'], copy=True)   # [256, 8192] u16 packed

    okey = ('out', length)
    if okey not in _CACHE:
        _CACHE[okey] = np.empty((length, BITS), np.float32)
        _CACHE['devg'] = np.empty((KOUT, BITS), np.uint16)
        _CACHE['btmp'] = np.empty((KOUT, BITS), np.uint16)
    out = _CACHE[okey]
    dev_g = _CACHE['devg']
    btmp = _CACHE['btmp']
    out[0] = out0
    colidx = place['slot_k'] * 64 + place['slot_n']
    np.take(dev, colidx, axis=1, out=dev_g)
    for v in range(16):
        rows = out[v + 1:length:16]
        nk = rows.shape[0]
        if nk:
            np.right_shift(dev_g[:nk], 15 - v, out=btmp[:nk])
            np.bitwise_and(btmp[:nk], 1, out=btmp[:nk])
            rows[:] = btmp[:nk]
    if _dbg:
        print(f"[kt] unpack {_time.perf_counter()-_tp:.3f}s", flush=True)
    return out


# revision 3
# speedup vs baseline: 1.2034x; 1.2034x over previous
"""AutoregressiveRAM kernel for trn2 — v3.

Changes vs v2:
- pos-bit contribution folded into PADD[t mod 16] (precomputed f32 table):
  no pos windows, no POSW, scalar engine out of the per-step loop.
- G1 AND+reduce runs on u32 pairs (DVE is 1 elem/cyc regardless of width,
  so pairing halves the dominant stage). W kept even for pairing.
- G2 one-hots in [s, w] layout (inner-dim stride-0 broadcast) so all
  reduces are contiguous (transposed reduces measured ~3x slower).
- DVE casts psum->swords and addr->addrI itself (engine chain DVE<->PE only).
"""
import sys
sys.path.insert(0, '/opt/trn_rl_repo')
import numpy as np

BITS = 8192
NB_T = 10
POS = 4
P = 128
S = 64
HI_W = 64
NSTEP = 4096
KOUT = NSTEP // 16

_CACHE = {}


def _color_and_place(conn, J, W, seed=0):
    """Pure-state greedy rainbow coloring + slot placement (pos taps excluded).

    Returns dict with color[8192], slot_k, slot_n, wordcol[8192], bitpos[8192].
    Window j = swords cols [j*W, (j+1)*W); pack column c maps to word
    (g=(c//64)%8, n=c%64): bits at partitions 16g..16g+15, column n.
    """
    rng = np.random.default_rng(seed)
    nbrs = [[] for _ in range(BITS)]
    for n in range(BITS):
        row = [b for b in conn[n] if b < BITS]
        for a in row:
            for b in row:
                if a != b:
                    nbrs[a].append(b)
    nbrs = [np.unique(np.array(x, dtype=np.int32)) if x else np.zeros(0, np.int32)
            for x in nbrs]

    C_s = J * W
    cover = [[] for _ in range(8 * 64)]
    col_of = {}
    for c in range(C_s):
        g, n = (c // 64) % 8, c % 64
        j = c // W
        cover[g * 64 + n].append(j)
        col_of[(g, n, j)] = c
    free = [[] for _ in range(J)]
    for g in range(8):
        for n in range(64):
            for j in cover[g * 64 + n]:
                for r in range(16):
                    free[j].append((16 * g + r, n))
    taken = np.zeros((P, 64), bool)
    free_ptr = [0] * J

    deg = np.array([len(x) for x in nbrs])
    order = np.argsort(-deg, kind='stable')
    color = np.full(BITS, -1, np.int32)
    slot_k = np.full(BITS, -1, np.int32)
    slot_n = np.full(BITS, -1, np.int32)
    wordcol = np.full(BITS, -1, np.int32)
    bitpos = np.full(BITS, -1, np.int32)
    cnt = np.zeros(J, np.int64)
    for q in order:
        forb = set(color[nbrs[q]].tolist()) if len(nbrs[q]) else set()
        forb.discard(-1)
        best, best_load = -1, None
        for j in rng.permutation(J):
            if j in forb:
                continue
            fl = free[j]
            while free_ptr[j] < len(fl) and taken[fl[free_ptr[j]][0], fl[free_ptr[j]][1]]:
                free_ptr[j] += 1
            if free_ptr[j] >= len(fl):
                continue
            if best == -1 or cnt[j] < best_load:
                best, best_load = j, cnt[j]
        if best == -1:
            return None
        j = best
        k, n = free[j][free_ptr[j]]
        free_ptr[j] += 1
        taken[k, n] = True
        color[q] = j
        slot_k[q] = k
        slot_n[q] = n
        wordcol[q] = col_of[(k // 16, n, j)]
        bitpos[q] = k % 16
        cnt[j] += 1

    if (color == -1).any():
        return None
    return dict(color=color, slot_k=slot_k, slot_n=slot_n,
                wordcol=wordcol, bitpos=bitpos)


def _build_constants(transition_memory, transition_connections, J, W, place):
    conn = transition_connections
    color = place['color']; slot_k = place['slot_k']; slot_n = place['slot_n']
    wordcol = place['wordcol']; bitpos = place['bitpos']
    C_s = -(-(J * W) // 64) * 64
    if C_s <= 512:
        C_s = 576

    neuron_at = np.full((P, S), -1, np.int64)
    neuron_at[slot_k, slot_n] = np.arange(BITS)
    assert (neuron_at >= 0).all()

    # tap descriptors for on-device expansion of ANDMASK / POWC
    TAPIDX = np.full((P, NB_T, S), J * W, np.uint16)   # jw index, OOR -> no-op
    TAPVAL = np.zeros((P, NB_T, S), np.uint16)         # 1 << bitpos
    TAPJ = np.full((P, NB_T, S), J, np.uint16)         # color, OOR -> no-op
    TAPW = np.zeros((P, NB_T, S), np.float64)          # 2^(9-jt) / 2^bitpos
    PADD = np.zeros((P, 16, S), np.float64)
    sk = slot_k; sn = slot_n
    for jt in range(NB_T):
        b = conn[:, jt]                                # [BITS] tap of each neuron
        wt = float(2 ** (NB_T - 1 - jt))
        is_pos = b >= BITS
        # pos taps -> PADD
        qpos = np.nonzero(is_pos)[0]
        for q in qpos:
            i = int(b[q]) - BITS
            for m in range(16):
                PADD[sk[q], m, sn[q]] += wt * ((m >> (3 - i)) & 1)
        qst = np.nonzero(~is_pos)[0]
        bb = b[qst]
        TAPIDX[sk[qst], jt, sn[qst]] = wordcol[bb]
        TAPVAL[sk[qst], jt, sn[qst]] = (1 << bitpos[bb]).astype(np.uint16)
        TAPJ[sk[qst], jt, sn[qst]] = color[bb]
        TAPW[sk[qst], jt, sn[qst]] = wt / (2.0 ** bitpos[bb])
    assert np.array_equal(TAPW.astype(np.float32).astype(np.float64), TAPW)

    cidx = np.arange(C_s)
    gofc = (cidx // 64) % 8
    MASKG = (np.arange(P)[:, None] // 16 == gofc[None, :]).astype(np.float32)
    PACKW = np.tile((2.0 ** (np.arange(P) % 16))[:, None], (1, P)).astype(np.float32)

    # TBLS u16 [P, S, HI_W]: packed table words in [s, w] layout
    T = transition_memory
    Tb = (T > 0.5).astype(np.uint16).reshape(BITS, HI_W, 16)
    U16 = (Tb << np.arange(16, dtype=np.uint16)[None, None, :]).sum(-1).astype(np.uint16)
    TBLS = U16[neuron_at, :]                    # [P, S, 64]

    return dict(TAPIDX=TAPIDX, TAPVAL=TAPVAL, TAPJ=TAPJ,
                TAPW=TAPW.astype(np.float32),
                PADD=PADD.astype(np.float32), MASKG=MASKG, PACKW=PACKW,
                TBLS=TBLS, C_s=C_s, neuron_at=neuron_at)


def _f32c(u16arr):
    a = np.ascontiguousarray(u16arr.reshape(P, -1))
    assert a.shape[1] % 2 == 0
    return a.view(np.float32)


def _bf16c(f32arr):
    import ml_dtypes
    a = np.ascontiguousarray(f32arr.reshape(P, -1)).astype(ml_dtypes.bfloat16)
    assert a.shape[1] % 2 == 0
    return a.view(np.uint16).view(np.float32)


def _build_program(J, W, C_s, n_steps=NSTEP):
    from concourse import bacc, mybir
    kout = n_steps // 16
    SJ = S * J
    JW = J * W
    assert W % 2 == 0
    nc = bacc.Bacc('TRN2', target_bir_lowering=False, debug=False)
    dt = mybir.dt

    def param(name, cols, dtype=dt.float32):
        return nc.declare_dram_parameter(name, [P, cols], dtype, isOutput=False)

    x_tapidx = param('x_tapidx', (NB_T * S) // 2)
    x_tapval = param('x_tapval', (NB_T * S) // 2)
    x_tapj = param('x_tapj', (NB_T * S) // 2)
    x_tapw = param('x_tapw', (NB_T * S) // 2)   # bf16 carrier
    x_padd = param('x_padd', 16 * S)
    x_packw = param('x_packw', P // 2)
    x_tbls = param('x_tbls', (S * HI_W) // 2)
    x_state0 = param('x_state0', S // 2)
    y = nc.declare_dram_parameter('y', [kout, BITS], dt.uint16, isOutput=True)

    from contextlib import ExitStack
    es = ExitStack()
    block = es.enter_context(nc.Block())
    sb = lambda name, cols, dtype: es.enter_context(nc.sbuf_tensor(name, [P, cols], dtype))
    ANDMASK = sb('ANDMASK', S * JW, dt.uint16)
    POWC = sb('POWC', SJ, dt.float32)
    PADD = sb('PADD', 16 * S, dt.float32)
    MASKG = sb('MASKG', C_s, dt.bfloat16)
    PACKW = sb('PACKW', P, dt.bfloat16)
    TAPIDX = sb('TAPIDX', NB_T * S, dt.uint16)
    TAPVAL = sb('TAPVAL', NB_T * S, dt.uint16)
    TAPJ = sb('TAPJ', NB_T * S, dt.uint16)
    TAPW = sb('TAPW', NB_T * S, dt.bfloat16)
    IOTAJW = sb('IOTAJW', JW, dt.uint16)
    IOTAJ = sb('IOTAJ', J, dt.uint16)
    GIOTA = sb('GIOTA', C_s, dt.uint16)
    PIDX = sb('PIDX', 2, dt.uint16)
    PIDXF = sb('PIDXF', 2, dt.float32)
    TBLS = sb('TBLS', S * HI_W, dt.uint16)
    IOTA_WS = sb('IOTA_WS', S * HI_W, dt.uint16)
    IOTA16S = sb('IOTA16S', S * 16, dt.uint16)
    POW2S = sb('POW2S', S * 16, dt.uint16)
    swords = sb('swords', JW, dt.uint16)
    bitsrep = sb('bitsrep', C_s, dt.bfloat16)
    tmp1 = sb('tmp1', S * JW, dt.uint16)
    sel32 = sb('sel32', 2 * SJ, dt.uint16)
    addr = sb('addr', S, dt.float32)
    addrI = sb('addrI', S, dt.uint16)
    hi6 = sb('hi6', S, dt.uint16)
    lo4 = sb('lo4', S, dt.uint16)
    word2 = sb('word2', S, dt.uint16)
    pw = sb('pw', S, dt.uint16)
    tb = sb('tb', S, dt.uint16)
    bitu = sb('bitu', S, dt.uint16)
    acc = sb('acc', S, dt.uint16)
    acc2 = sb('acc2', S, dt.uint16)
    stage = sb('stage', S, dt.uint16)
    psum = es.enter_context(nc.psum_tensor('psum', [P, C_s], dt.float32))

    s_in = es.enter_context(nc.semaphore('s_in'))
    s_cg = es.enter_context(nc.semaphore('s_cg'))
    s_rhs = es.enter_context(nc.semaphore('s_rhs'))
    s_pe = es.enter_context(nc.semaphore('s_pe'))
    s_bf = es.enter_context(nc.semaphore('s_bf'))
    s_dma = es.enter_context(nc.semaphore('s_dma'))

    N_IN_DMAS = 8

    AND = mybir.AluOpType.bitwise_and
    OR = mybir.AluOpType.bitwise_or
    MULT = mybir.AluOpType.mult
    ADD = mybir.AluOpType.add
    EQ = mybir.AluOpType.is_equal
    SHR = mybir.AluOpType.logical_shift_right
    SHL = mybir.AluOpType.logical_shift_left
    X = mybir.AxisListType.X

    import concourse.bass as bass_mod
    ds = bass_mod.ds

    @block.sync
    def _(sync):
        for t_, src in [(TAPIDX, x_tapidx), (TAPVAL, x_tapval), (TAPJ, x_tapj),
                        (TAPW, x_tapw), (PADD, x_padd), (PACKW, x_packw),
                        (TBLS, x_tbls), (bitu, x_state0)]:
            sync.dma_start(out=t_[:], in_=src[:].bitcast(t_.dtype)).then_inc(s_in, 16)
        with sync.Fori(0, kout) as k:
            sync.wait_ge(s_bf, k + 1)
            sync.dma_start(
                out=y[ds(k, 1), :],
                in_=stage[:].unsqueeze(1),
            ).then_inc(s_dma, 16)

    @block.gpsimd
    def _(gpsimd):
        gpsimd.iota(IOTA_WS[:].rearrange('p (s w) -> p s w', s=S),
                    pattern=[[0, S], [1, HI_W]], base=0, channel_multiplier=0)
        gpsimd.iota(IOTA16S[:].rearrange('p (s b) -> p s b', s=S),
                    pattern=[[0, S], [1, 16]], base=0, channel_multiplier=0)
        gpsimd.iota(IOTAJW[:], pattern=[[1, JW]], base=0, channel_multiplier=0)
        gpsimd.iota(IOTAJ[:], pattern=[[1, J]], base=0, channel_multiplier=0)
        gpsimd.iota(GIOTA[:].rearrange('p (a b) -> p a b', b=64),
                    pattern=[[1, C_s // 64], [0, 64]], base=0, channel_multiplier=0)
        gpsimd.iota(PIDX[:], pattern=[[0, 2]], base=0, channel_multiplier=1)
        gpsimd.memset(POW2S[:], 1)
        gpsimd.memset(acc[:], 0)
        gpsimd.memset(ANDMASK[:], 0)
        gpsimd.memset(POWC[:], 0)
        gpsimd.drain().then_inc(s_cg, 1)

    @block.tensor
    def _(tensor):
        tensor.wait_ge(s_in, 16 * N_IN_DMAS)
        with tensor.Fori(0, n_steps, 16) as base:
            for u in range(16):
                tensor.wait_ge(s_rhs, base + (u + 1))
                tensor.matmul(psum[:, 0:512], PACKW[:], bitsrep[:, 0:512],
                              start=True, stop=True)
                tensor.matmul(psum[:, 512:C_s], PACKW[:], bitsrep[:, 512:C_s],
                              start=True, stop=True).then_inc(s_pe, 1)

    @block.vector
    def _(vector):
        _lp = nc.allow_low_precision(reason='u16/u32 one-hot reductions, exact by construction')
        _lp.__enter__()
        vector.wait_ge(s_in, 16 * N_IN_DMAS)
        vector.wait_ge(s_cg, 1)
        vector.tensor_tensor(POW2S[:], POW2S[:], IOTA16S[:], SHL)
        vector.drain()
        # MASKG = (p//16 == (c//64)%8) in bf16
        vector.tensor_scalar(GIOTA[:], GIOTA[:], 7, None, AND)
        vector.tensor_scalar(PIDX[:], PIDX[:], 4, None, SHR)
        vector.drain()
        vector.tensor_scalar(PIDXF[:], PIDX[:], 0, None, ADD)
        vector.drain()
        vector.tensor_scalar(MASKG[:], GIOTA[:], PIDXF[:, 0:1], None, EQ)
        vector.drain()
        # expand ANDMASK from tap descriptors
        tmp1v = tmp1[:].rearrange('p (s c) -> p s c', s=S)
        for t in range(NB_T):
            vector.tensor_tensor(
                tmp1v,
                TAPIDX[:].rearrange('p (t s) -> p t s', t=NB_T)[:, t, :].unsqueeze(2).broadcast_to([P, S, JW]),
                IOTAJW[:].unsqueeze(1).broadcast_to([P, S, JW]),
                EQ)
            vector.drain()
            vector.tensor_tensor(
                tmp1v, tmp1v,
                TAPVAL[:].rearrange('p (t s) -> p t s', t=NB_T)[:, t, :].unsqueeze(2).broadcast_to([P, S, JW]),
                MULT)
            vector.drain()
            vector.tensor_tensor(ANDMASK[:].rearrange('p (s c) -> p s c', s=S),
                                 ANDMASK[:].rearrange('p (s c) -> p s c', s=S),
                                 tmp1v, OR)
            vector.drain()
        # expand POWC from tap descriptors (reuses tmp1 scratch; one-hot in f32
        # so the TAPW multiply is a f32 x bf16 op)
        ohjf = tmp1[:, 0:2 * SJ].bitcast(dt.float32).rearrange('p (s j) -> p s j', s=S)
        tmpf = tmp1[:, 2 * SJ:4 * SJ].bitcast(dt.float32).rearrange('p (s j) -> p s j', s=S)
        for t in range(NB_T):
            vector.tensor_tensor(
                ohjf,
                TAPJ[:].rearrange('p (t s) -> p t s', t=NB_T)[:, t, :].unsqueeze(2).broadcast_to([P, S, J]),
                IOTAJ[:].unsqueeze(1).broadcast_to([P, S, J]),
                EQ)
            vector.drain()
            vector.tensor_tensor(
                tmpf, ohjf,
                TAPW[:].rearrange('p (t s) -> p t s', t=NB_T)[:, t, :].unsqueeze(2).broadcast_to([P, S, J]),
                MULT)
            vector.drain()
            vector.tensor_tensor(POWC[:].rearrange('p (s j) -> p s j', s=S),
                                 POWC[:].rearrange('p (s j) -> p s j', s=S),
                                 tmpf, ADD)
            vector.drain()
        # prologue: bitsrep for it=1 from state0 (in bitu)
        vector.tensor_tensor(bitsrep[:].rearrange('p (r n) -> p r n', n=S),
                             bitu[:].unsqueeze(1).broadcast_to([P, C_s // S, S]),
                             MASKG[:].rearrange('p (r n) -> p r n', n=S),
                             MULT).then_inc(s_rhs, 1)
        vector.drain()

        W2 = W // 2
        tmp32 = tmp1[:].bitcast(dt.uint32)       # [P, S*JW/2] u32
        sel32v = sel32[:].bitcast(dt.uint32)     # [P, SJ] u32
        sw32 = swords[:].bitcast(dt.uint32)      # [P, JW/2]
        am32 = ANDMASK[:].bitcast(dt.uint32)
        # scratch carved from tmp1 (free once the G1 reduce lands in sel32)
        o = 0
        selh = tmp1[:, o:o + 2 * SJ]; o += 2 * SJ
        selhv = selh.bitcast(dt.uint32)
        sel2 = tmp1[:, o:o + SJ]; o += SJ
        if o % 2:
            o += 1
        addrt = tmp1[:, o:o + 2 * SJ].bitcast(dt.float32); o += 2 * SJ
        onehot = tmp1[:, o:o + S * HI_W]; o += S * HI_W
        wtmp = tmp1[:, o:o + S * HI_W]; o += S * HI_W
        oh16 = tmp1[:, o:o + S * 16]; o += S * 16
        ptmp = tmp1[:, o:o + S * 16]; o += S * 16
        assert o <= S * JW, (o, S * JW)

        with vector.Fori(0, n_steps, 16) as base:
            for u in range(16):
                m = (u + 1) & 15
                vector.wait_ge(s_pe, base + (u + 1))
                # cast psum words -> u16 swords
                vector.tensor_scalar(swords[:], psum[:, 0:JW], 0, None, ADD)
                vector.drain()
                # G1 on u32 pairs
                vector.tensor_tensor(
                    tmp32.rearrange('p (s j c) -> p s j c', s=S, j=J),
                    sw32.rearrange('p (j c) -> p j c', j=J).unsqueeze(1).broadcast_to([P, S, J, W2]),
                    am32.rearrange('p (s j c) -> p s j c', s=S, j=J),
                    AND)
                vector.drain()
                vector.tensor_reduce(sel32v.rearrange('p (s j) -> p s j', s=S),
                                     tmp32.rearrange('p (s j c) -> p s j c', s=S, j=J),
                                     axis=X, op=ADD)
                vector.drain()
                # merge u16 lanes: sel2 = lo + hi (exactly one is nonzero)
                vector.tensor_scalar(selhv, sel32v, 16, None, SHR)
                vector.drain()
                vector.tensor_tensor(selhv, sel32v, selhv, ADD)
                vector.drain()
                vector.tensor_scalar(selhv, selhv, 0xFFFF, None, AND)
                vector.drain()
                vector.tensor_scalar(sel2, selhv, 0, None, ADD)  # u32 -> u16
                vector.drain()
                # addr = sum_j sel2 * POWC + PADD[m]
                vector.tensor_tensor(addrt.rearrange('p (s j) -> p s j', s=S),
                                     sel2.rearrange('p (s j) -> p s j', s=S),
                                     POWC[:].rearrange('p (s j) -> p s j', s=S),
                                     MULT)
                vector.drain()
                vector.tensor_reduce(addr[:].rearrange('p s -> p s'),
                                     addrt.rearrange('p (s j) -> p s j', s=S),
                                     axis=X, op=ADD)
                vector.drain()
                vector.tensor_tensor(addr[:], addr[:],
                                     PADD[:].rearrange('p (m s) -> p m s', m=16)[:, m, :],
                                     ADD)
                vector.drain()
                vector.tensor_scalar(addrI[:], addr[:], 0, None, ADD)  # f32 -> u16
                vector.drain()
                vector.tensor_scalar(hi6[:], addrI[:], 4, None, SHR)
                vector.tensor_scalar(lo4[:], addrI[:], 15, None, AND)
                vector.drain()
                # G2 in [s, w] layout, contiguous reduces
                vector.tensor_tensor(onehot.rearrange('p (s w) -> p s w', s=S),
                                     hi6[:].unsqueeze(2).broadcast_to([P, S, HI_W]),
                                     IOTA_WS[:].rearrange('p (s w) -> p s w', s=S),
                                     EQ)
                vector.tensor_tensor(oh16.rearrange('p (s b) -> p s b', s=S),
                                     lo4[:].unsqueeze(2).broadcast_to([P, S, 16]),
                                     IOTA16S[:].rearrange('p (s b) -> p s b', s=S),
                                     EQ)
                vector.drain()
                vector.tensor_tensor(wtmp.rearrange('p (s w) -> p s w', s=S),
                                     onehot.rearrange('p (s w) -> p s w', s=S),
                                     TBLS[:].rearrange('p (s w) -> p s w', s=S),
                                     MULT)
                vector.tensor_tensor(ptmp.rearrange('p (s b) -> p s b', s=S),
                                     oh16.rearrange('p (s b) -> p s b', s=S),
                                     POW2S[:].rearrange('p (s b) -> p s b', s=S),
                                     MULT)
                vector.drain()
                vector.tensor_reduce(word2[:].rearrange('p s -> p s'),
                                     wtmp.rearrange('p (s w) -> p s w', s=S),
                                     axis=X, op=ADD)
                vector.tensor_reduce(pw[:].rearrange('p s -> p s'),
                                     ptmp.rearrange('p (s b) -> p s b', s=S),
                                     axis=X, op=ADD)
                vector.drain()
                vector.tensor_tensor(tb[:], word2[:], pw[:], AND)
                vector.drain()
                vector.tensor_scalar(bitu[:], tb[:], 0, None, mybir.AluOpType.is_gt)
                vector.drain()
                # next-iteration rhs (unblocks PE) + acc update
                vector.tensor_tensor(bitsrep[:].rearrange('p (r n) -> p r n', n=S),
                                     bitu[:].unsqueeze(1).broadcast_to([P, C_s // S, S]),
                                     MASKG[:].rearrange('p (r n) -> p r n', n=S),
                                     MULT).then_inc(s_rhs, 1)
                vector.tensor_scalar(acc2[:], acc[:], 0x7fff, 1, AND, SHL)
                vector.drain()
                vector.tensor_tensor(acc[:], acc2[:], bitu[:], OR)
                vector.drain()
            vector.wait_ge(s_dma, base)
            vector.tensor_scalar(stage[:], acc[:], 0, None, ADD).then_inc(s_bf, 1)
            vector.drain()
        _lp.__exit__(None, None, None)

    es.close()
    nc.finalize()
    return nc


def kernel(transition_memory, initial_memory, transition_connections,
           initial_connections, length):
    from concourse.bass_utils import run_bass_kernel_spmd
    import time as _time
    import os as _os
    _dbg = _os.environ.get('KERNEL_TIMERS')
    length = int(length)
    conn = np.asarray(transition_connections)
    if 'prog' not in _CACHE:
        place = None
        for (J, W, seed) in [(38, 14, 0), (39, 14, 1), (40, 14, 0), (42, 14, 0)]:
            place = _color_and_place(conn, J, W, seed=seed)
            if place is not None:
                break
        assert place is not None, 'coloring failed'
        consts = _build_constants(np.asarray(transition_memory), conn, J, W, place)
        prog = _build_program(J, W, consts['C_s'], NSTEP)
        _CACHE['prog'] = (J, W, place, consts, prog)
    J, W, place, consts, prog = _CACHE['prog']

    # step 0 host-side
    ic = np.asarray(initial_connections)
    im = np.asarray(initial_memory)
    addr0 = (np.zeros(POS, np.int64)[ic] * (2 ** np.arange(3, -1, -1))).sum(1)
    out0 = im[np.arange(BITS), addr0].astype(np.float32)
    st0 = (out0 > 0.5).astype(np.uint16)[consts['neuron_at']]

    _tp = _time.perf_counter()
    ins = {
        'x_tapidx': _f32c(consts['TAPIDX']),
        'x_tapval': _f32c(consts['TAPVAL']),
        'x_tapj': _f32c(consts['TAPJ']),
        'x_tapw': _bf16c(consts['TAPW']),
        'x_padd': np.ascontiguousarray(consts['PADD'].reshape(P, -1)),
        'x_packw': _bf16c(consts['PACKW']),
        'x_tbls': _f32c(consts['TBLS']),
        'x_state0': _f32c(st0),
    }
    if _dbg:
        print(f"[kt] ins prep {_time.perf_counter()-_tp:.3f}s", flush=True)
    _t0 = _time.perf_counter()
    res = run_bass_kernel_spmd(prog, [ins], core_ids=[0])
    global LAST_EXEC_NS
    LAST_EXEC_NS = res.exec_time_ns
    if LAST_EXEC_NS is None:
        LAST_EXEC_NS = int((_time.perf_counter() - _t0) * 1e9)
    if _dbg:
        print(f"[kt] run_bass {_time.perf_counter()-_t0:.3f}s", flush=True)
    _tp = _time.perf_counter()
    dev = np.array(res.results[0]['y'], copy=True)   # [256, 8192] u16 packed

    okey = ('out', length)
    if okey not in _CACHE:
        _CACHE[okey] = np.empty((length, BITS), np.float32)
        _CACHE['devg'] = np.empty((KOUT, BITS), np.uint16)
        _CACHE['btmp'] = np.empty((KOUT, BITS), np.uint16)
    out = _CACHE[okey]
    dev_g = _CACHE['devg']
    btmp = _CACHE['btmp']
    out[0] = out0
    colidx = place['slot_k'] * 64 + place['slot_n']
    np.take(dev, colidx, axis=1, out=dev_g)
    for v in range(16):
        rows = out[v + 1:length:16]
        nk = rows.shape[0]
        if nk:
            np.right_shift(dev_g[:nk], 15 - v, out=btmp[:nk])
            np.bitwise_and(btmp[:nk], 1, out=btmp[:nk])
            rows[:] = btmp[:nk]
    if _dbg:
        print(f"[kt] unpack {_time.perf_counter()-_tp:.3f}s", flush=True)
    return out
